# revision 1
# baseline (speedup 1.0000x reference)
"""Bidirectional LSTM layer (T=1024, B=64, I=H=512) on 8 Trainium2 NeuronCores.

Sharding: cores 0-3 forward direction, cores 4-7 backward; each core owns 16
batch columns and runs the full sequential scan.

Key structure (v2):
- All raggedness (flip_batch) handled host-side: x is pre-flipped/pre-packed,
  outputs are written in scan order and un-flipped on the host. No indirect DMA.
- x is host-pretransposed into PE-ready [I, rows] tiles (no on-device x
  transposes).
- All-sigmoid trick: tanh(x) = 2*sigmoid(2x) - 1 folded into host weight
  scaling; h is kept as h/2 on device (un-scaled on host). Every LUT op is a
  Sigmoid, fused tails use scalar_tensor_tensor.
- W_hh recurrence matmuls use fp8e4m3 DoubleRow (2 k-pairs, 0.5 cyc/row) or
  bf16 (4 k-tiles); x-projection GEMM is bf16, batched 8 steps at a time, and
  injected into the gate PSUM via identity('sel') matmuls.
- Output accumulates in an SBUF ring, one contiguous DMA per 8 steps.
"""

import sys
from contextlib import ExitStack

import numpy as np
import ml_dtypes

for p in ("/opt/trn_rl_repo", "/root/.axon_site/_ro/trn_rl_repo"):
    if p not in sys.path:
        sys.path.append(p)

import concourse.bass as bass
import concourse.tile as tile
from concourse import bacc, mybir
from concourse.bass_utils import run_bass_kernel_spmd
from concourse.masks import make_identity

F32 = mybir.dt.float32
F32R = mybir.dt.float32r
BF16 = mybir.dt.bfloat16
F8 = mybir.dt.float8e4
F16 = mybir.dt.float16

T, B, I, H = 1024, 64, 512, 512
G = 4 * H
NCORES = 8
BL = B // (NCORES // 2)       # 16 batch columns per core
SB = 8                        # steps per x/out block (128 rows)
HH = H // 2                   # half-width for pipelined tail

WHH_MODE = "fp16"             # recurrence weight dtype
KEEP1 = 3                    # p-state keeper matmuls per step

# gate chunk order in reordered weights: (g, f, i, o); torch order is (i,f,g,o)
_PERM = np.concatenate([
    np.arange(2 * H, 3 * H),  # g
    np.arange(0, H),          # i
    np.arange(H, 2 * H),      # f
    np.arange(3 * H, 4 * H),  # o
])
CG, CI, CF, CO = 0, 1, 2, 3   # chunk indices

AF = mybir.ActivationFunctionType
ALU = mybir.AluOpType
DR = mybir.MatmulPerfMode.DoubleRow


def build_program(t_steps=T):
    assert t_steps % SB == 0
    nblk = t_steps // SB
    whh_dt = F16
    nc = bacc.Bacc("TRN2", target_bir_lowering=False, debug=False)

    xT_d = nc.dram_tensor("xT", [nblk * 128, I], F16, kind="ExternalInput").ap()
    wih_d = nc.dram_tensor("wihT", [I, G], F16, kind="ExternalInput").ap()
    whh_d = nc.dram_tensor("whhT", [H, G], whh_dt, kind="ExternalInput").ap()
    bias_d = nc.dram_tensor("bias", [1, G], F16, kind="ExternalInput").ap()
    hT0_d = nc.dram_tensor("hT0", [128, 4 * BL], whh_dt, kind="ExternalInput").ap()
    c0_d = nc.dram_tensor("c0", [BL, H], F16, kind="ExternalInput").ap()
    out_d = nc.dram_tensor("out", [t_steps * BL, H], F16, kind="ExternalOutput").ap()

    with tile.TileContext(nc) as tc, ExitStack() as ctx:
        cpool = ctx.enter_context(tc.tile_pool(name="consts", bufs=1))
        ident_f = cpool.tile([128, 128], F32, tag="identf")
        make_identity(nc, ident_f[:])
        ident = cpool.tile([128, 128], F16, tag="ident")
        nc.vector.tensor_copy(ident[:], ident_f[:])
        ones_f = cpool.tile([1, 128], F32, tag="onesf")
        nc.vector.memset(ones_f[:], 1.0)
        ones_r = cpool.tile([1, 128], F16, tag="ones")
        nc.vector.tensor_copy(ones_r[:], ones_f[:])

        wih_sb = cpool.tile([128, 4 * G], F16, tag="wih")    # [k]*G + col
        whh_sb = cpool.tile([128, 4 * G], whh_dt, tag="whh")  # [k]*G + col
        for k in range(4):
            nc.sync.dma_start(wih_sb[:, k * G:(k + 1) * G],
                              wih_d[k * 128:(k + 1) * 128, :])
            nc.sync.dma_start(whh_sb[:, k * G:(k + 1) * G],
                              whh_d[k * 128:(k + 1) * 128, :])
        bias_sb = cpool.tile([1, G], F16, tag="bias")
        nc.sync.dma_start(bias_sb[:], bias_d[:])
        bias_full = cpool.tile([128, G], F16, tag="biasfull")

        # persistent state
        hT = [cpool.tile([128, 4 * BL], whh_dt, tag=f"hT{i}", name=f"hT{i}") for i in range(2)]
        nc.sync.dma_start(hT[0][:], hT0_d[:])
        c_st = cpool.tile([BL, H], F16, tag="c")
        nc.sync.dma_start(c_st[:], c0_d[:])

        # double-buffered work tiles
        sg = [cpool.tile([BL, H], F16, tag=f"sg{i}", name=f"sg{i}") for i in range(2)]
        sf = [cpool.tile([BL, H], F16, tag=f"sf{i}", name=f"sf{i}") for i in range(2)]
        si = [cpool.tile([BL, H], F16, tag=f"si{i}", name=f"si{i}") for i in range(2)]
        so = [cpool.tile([BL, H], F16, tag=f"so{i}", name=f"so{i}") for i in range(2)]
        q_sb = cpool.tile([BL, H], F16, tag="q")
        m1_sb = cpool.tile([BL, H], F16, tag="m1")
        tc_sb = cpool.tile([BL, H], F16, tag="tc")
        xp_sb = [cpool.tile([128, G], F16, tag=f"xp{i}", name=f"xp{i}") for i in range(2)]
        xT_sb = [cpool.tile([128, I], F16, tag=f"xT{i}", name=f"xTs{i}") for i in range(2)]
        ring = [cpool.tile([BL, SB * H], F16, tag=f"ring{i}", name=f"ring{i}") for i in range(2)]

        ps_g_pool = ctx.enter_context(tc.tile_pool(name="psg", bufs=1, space="PSUM"))
        ps_bg_pool = ctx.enter_context(tc.tile_pool(name="psbg", bufs=1, space="PSUM"))
        ps_t_pool = ctx.enter_context(tc.tile_pool(name="pst", bufs=1, space="PSUM"))
        ps_k_pool = ctx.enter_context(tc.tile_pool(name="psk", bufs=1, space="PSUM"))
        ps_c = [ps_g_pool.tile([BL, 512], F32, tag=f"psc{n}", name=f"psc{n}")
                for n in range(4)]
        ps_bg = ps_bg_pool.tile([128, 1024], F32, tag="psbg")
        psT = ps_t_pool.tile([128, 4 * BL], F16, tag="psT")
        ps_keep = ps_k_pool.tile([128, 512], F32, tag="pskeep")

        def xp_gemm_half(buf, half):
            """Project chunks (half*2, half*2+1) of xT_sb[buf] into ps_bg."""
            for j in range(2):
                n = half * 2 + j
                ncols = slice(j * 512, (j + 1) * 512)
                for k in range(4):
                    nc.tensor.matmul(
                        out=ps_bg[:, ncols],
                        lhsT=xT_sb[buf][:, k * 128:(k + 1) * 128],
                        rhs=wih_sb[:, k * G + n * 512:k * G + (n + 1) * 512],
                        start=(k == 0), stop=(k == 3),
                    )

        def xp_copy_half(buf, half):
            for j in range(2):
                n = half * 2 + j
                nc.vector.scalar_tensor_tensor(
                    out=xp_sb[buf][:, n * 512:(n + 1) * 512],
                    in0=ps_bg[:, j * 512:(j + 1) * 512],
                    scalar=1.0,
                    in1=bias_full[:, n * 512:(n + 1) * 512],
                    op0=ALU.bypass, op1=ALU.add,
                )

        def rec_matmuls(s, cur):
            l = s % SB
            par = s % 2
            sel = ident[:, l * BL:(l + 1) * BL]
            for n in range(4):
                nc.tensor.matmul(
                    out=ps_c[n][:BL, :],
                    lhsT=sel,
                    rhs=xp_sb[cur][:, n * 512:(n + 1) * 512],
                    start=True, stop=False,
                )
                for k in range(4):
                    nc.tensor.matmul(
                        out=ps_c[n][:BL, :],
                        lhsT=hT[par][:, k * BL:(k + 1) * BL],
                        rhs=whh_sb[:, k * G + n * 512:k * G + (n + 1) * 512],
                        start=False, stop=(k == 3),
                    )

        def keepers(n):
            for _ in range(n):
                nc.tensor.matmul(
                    out=ps_keep[:],
                    lhsT=ident[:, :128],
                    rhs=wih_sb[:, :512],
                    start=True, stop=True,
                )

        def step_tail(s):
            """Activations + elementwise for step s; writes ring."""
            l = s % SB
            par = s % 2
            rbuf = (s // SB) % 2
            g_, f_, i_, o_ = sg[par], sf[par], si[par], so[par]
            nc.scalar.activation(g_[:], ps_c[CG][:BL, :], AF.Sigmoid)
            nc.scalar.activation(i_[:], ps_c[CI][:BL, :], AF.Sigmoid)
            nc.scalar.activation(f_[:], ps_c[CF][:BL, :], AF.Sigmoid)
            nc.scalar.activation(o_[:], ps_c[CO][:BL, :], AF.Sigmoid)
            # DVE tail: q = (sig(g')-0.5)*sig(i); m1 = sig(f)*c; c = 2q+m1
            for h in range(2):
                hs = slice(h * HH, (h + 1) * HH)
                nc.vector.scalar_tensor_tensor(
                    out=q_sb[:, hs], in0=g_[:, hs], scalar=0.5, in1=i_[:, hs],
                    op0=ALU.subtract, op1=ALU.mult)
            for h in range(2):
                hs = slice(h * HH, (h + 1) * HH)
                nc.vector.tensor_mul(m1_sb[:, hs], f_[:, hs], c_st[:, hs])
                nc.vector.scalar_tensor_tensor(
                    out=c_st[:, hs], in0=q_sb[:, hs], scalar=2.0, in1=m1_sb[:, hs],
                    op0=ALU.mult, op1=ALU.add)
            for h in range(2):
                hs = slice(h * HH, (h + 1) * HH)
                nc.scalar.activation(tc_sb[:, hs], c_st[:, hs], AF.Sigmoid, scale=2.0)
            for h in range(2):
                hs = slice(h * HH, (h + 1) * HH)
                nc.vector.scalar_tensor_tensor(
                    out=ring[rbuf][:, l * H + h * HH:l * H + (h + 1) * HH],
                    in0=tc_sb[:, hs], scalar=-0.5, in1=o_[:, hs],
                    op0=ALU.add, op1=ALU.mult)

        def step_transpose(s):
            l = s % SB
            rbuf = (s // SB) % 2
            nxt = (s + 1) % 2
            for half in range(2):
                for k in (2 * half, 2 * half + 1):
                    nc.tensor.transpose(
                        out=psT[:, k * BL:(k + 1) * BL],
                        in_=ring[rbuf][:BL, l * H + k * 128:l * H + (k + 1) * 128],
                        identity=ident[:BL, :BL],
                    )
                nc.scalar.copy(
                    hT[nxt][:, half * 2 * BL:(half + 1) * 2 * BL],
                    psT[:, half * 2 * BL:(half + 1) * 2 * BL])

        def flush_out(blk):
            rbuf = blk % 2
            nc.sync.dma_start(
                out_d[blk * 128:(blk + 1) * 128, :].rearrange(
                    "(l b) c -> b l c", l=SB),
                ring[rbuf][:BL, :].rearrange("p (l c) -> p l c", l=SB),
            )

        # broadcast bias to all 128 partitions (once)
        for n in range(4):
            nc.tensor.matmul(
                out=ps_bg[:, :512], lhsT=ones_r[:1, :128],
                rhs=bias_sb[:1, n * 512:(n + 1) * 512], start=True, stop=True)
            nc.scalar.copy(bias_full[:, n * 512:(n + 1) * 512], ps_bg[:, :512])
        # prologue: block 0 xp
        nc.sync.dma_start(xT_sb[0][:], xT_d[0:128, :])
        for half in range(2):
            xp_gemm_half(0, half)
            xp_copy_half(0, half)

        for s in range(t_steps):
            l = s % SB
            blk = s // SB
            cur = blk % 2
            nxt = 1 - cur
            have_next = blk + 1 < nblk

            rec_matmuls(s, cur)
            # interleave next block's xp pipeline
            if have_next:
                if l == 0:
                    nc.sync.dma_start(
                        xT_sb[nxt][:], xT_d[(blk + 1) * 128:(blk + 2) * 128, :])
                elif l == 1:
                    xp_gemm_half(nxt, 0)
                elif l == 4:
                    xp_gemm_half(nxt, 1)
            step_tail(s)
            step_transpose(s)
            if have_next and l in (2, 5):
                xp_copy_half(nxt, (l - 2) // 3)
            keepers(1 if (have_next and l in (1, 4)) else KEEP1)
            if l == SB - 1:
                flush_out(blk)

    nc.compile()
    return nc


def _flip_idx(lengths, t_steps):
    t = np.arange(t_steps, dtype=np.int64)[:, None]
    L = lengths[None, :]
    return np.where(t < L, L - 1 - t, t)             # [T, BL]


def make_core_inputs(inputs, core, t_steps=T):
    fwd = core < NCORES // 2
    sl = slice((core % 4) * BL, (core % 4) * BL + BL)
    whh_np = np.float16
    x = np.asarray(inputs["input"], np.float32)[:t_steps, sl, :]
    lengths = np.asarray(inputs["lengths"]).astype(np.int64)[sl]
    sfx = "f" if fwd else "b"
    w_ih = np.asarray(inputs[f"w_ih_{sfx}"], np.float32).copy()
    w_hh = np.asarray(inputs[f"w_hh_{sfx}"], np.float32).copy()
    bias = (np.asarray(inputs[f"b_ih_{sfx}"], np.float32)
            + np.asarray(inputs[f"b_hh_{sfx}"], np.float32)).copy()
    h0 = np.asarray(inputs[f"h0_{sfx}"], np.float32)[sl]
    c0 = np.asarray(inputs[f"c0_{sfx}"], np.float32)[sl]

    # all-sigmoid scaling: g-gate pre-activation x2; recurrent weights see
    # h/2 on device so W_hh x2 (g rows x4)
    gsl = slice(2 * H, 3 * H)
    w_ih[gsl] *= 2.0
    bias[gsl] *= 2.0
    w_hh *= 2.0
    w_hh[gsl] *= 2.0

    if not fwd:
        idx = _flip_idx(lengths, t_steps)
        x = np.take_along_axis(x, idx[:, :, None], axis=0)

    nblk = t_steps // SB
    # xT: [blk, p(I%128), k(I//128), s_local, b]
    xs = x.reshape(nblk, SB, BL, 4, 128)
    xT = np.ascontiguousarray(xs.transpose(0, 4, 3, 1, 2)).reshape(nblk * 128, I)

    hT0 = (h0 / 2.0).T.reshape(4, 128, BL).transpose(1, 0, 2).reshape(128, 4 * BL)

    return {
        "xT": xT.astype(np.float16),
        "wihT": np.ascontiguousarray(w_ih.T[:, _PERM]).astype(np.float16),
        "whhT": np.ascontiguousarray(w_hh.T[:, _PERM]).astype(whh_np),
        "bias": np.ascontiguousarray(bias[_PERM][None, :]).astype(np.float16),
        "hT0": np.ascontiguousarray(hT0).astype(whh_np),
        "c0": np.ascontiguousarray(c0).astype(np.float16),
    }


def assemble_output(inputs, results, t_steps=T):
    """Combine per-core 'out' arrays (scan-order h/2 in bf16) into [T,B,2H]."""
    out = np.empty((t_steps, B, 2 * H), np.float32)
    lengths_all = np.asarray(inputs["lengths"]).astype(np.int64)
    for c in range(NCORES):
        sl = slice((c % 4) * BL, (c % 4) * BL + BL)
        ys = 2.0 * np.asarray(results[c]["out"]).astype(np.float32).reshape(
            t_steps, BL, H)
        if c < NCORES // 2:
            out[:, sl, :H] = ys
        else:
            idx = _flip_idx(lengths_all[sl], t_steps)
            out[:, sl, H:] = np.take_along_axis(ys, idx[:, :, None], axis=0)
    return out


_PROGRAM_CACHE = {}


def kernel(**inputs) -> np.ndarray:
    t_steps = inputs["input"].shape[0]
    if t_steps not in _PROGRAM_CACHE:
        _PROGRAM_CACHE[t_steps] = build_program(t_steps)
    nc = _PROGRAM_CACHE[t_steps]
    in_maps = [make_core_inputs(inputs, c, t_steps) for c in range(NCORES)]
    res = run_bass_kernel_spmd(nc, in_maps, list(range(NCORES)))
    return assemble_output(inputs, res.results, t_steps)


if __name__ == "__main__":
    pass

